# revision 12
# baseline (speedup 1.0000x reference)
"""Trainium2 Bass kernel for the SNN Leaky-Integrate-Fire problem.

Pipeline (per core, pure data-parallel over batch):
  cur1 = x @ W1.T + b1                        [B,32]  (PE fp32 matmul, bit-exact
                                                       vs the XLA-neuron reference)
  100x: mem = beta*mem + cur1 - H(mem-1)      (elementwise scan; 2 fused STT ops/step)
  spk  = H(mem - 1)
  out  = spk @ W2.T + b2                      [B,3]   (segmented reduce over h)

Numerics: the scan tracks n = -mem so each step is exactly two
scalar_tensor_tensor ops whose per-stage fp32 roundings match the
reference's  fl(fl(fl(beta*m)+c)-h)  sequence bit-for-bit:
  A  = (n * -beta) - cn         # cn = -cur1; A = fl(fl(beta*m)+cur1)
  n' = (n is_lt -1) - A         # n' = fl(h - A) = -m'

cur1 is computed on the PE with the exact operand layout the XLA-neuron
compiler uses for this matmul (stationary = x-chunk.T [3,128], moving =
W1.T [3,32], fp32 LOW/HIGH mode) -> bit-identical spikes.

Layout per core: 8192 rows; logical row r = chunk*128 + p lives at
partition p, free block chunk. Host feeds x_shard.T [3, 8192] and
inverse-permutes the output rows.
"""
import os
import sys

sys.path.insert(0, "/opt/trn_rl_repo")

import numpy as np

import concourse.bacc as bacc
import concourse.tile as tile
from concourse import mybir
from concourse.bass_utils import run_bass_kernel_spmd

F32 = mybir.dt.float32
ALU = mybir.AluOpType

# problem constants (hardcoded per contract)
B, N_IN, N_HID, N_OUT = 65536, 3, 32, 3
NUM_STEPS, BETA, THR = 100, 0.9, 1.0
N_CORES = 8
BC = B // N_CORES          # rows per core = 8192
P = 128                    # partitions
NCH = BC // P              # 128-row chunks per core = 64
FREE = NCH * N_HID         # scan free size = 2048

# const block layout (replicated across partitions): [b1(32) w2(3*32) b2(3)]
B1_OFF, W2_OFF, B2_OFF = 0, 32, 128
WB_COLS = 131

# scan columns handled by GPSIMD (0 = DVE only); must be a multiple of N_HID.
GP_COLS = int(os.environ.get("KERNEL_GP_COLS", "512"))


def build(nc, n_rows_core=BC, num_steps=NUM_STEPS, gp_cols=GP_COLS):
    nch = n_rows_core // P
    free = nch * N_HID
    xt_d = nc.dram_tensor("xT", [N_IN, n_rows_core], F32, kind="ExternalInput")
    w1t_d = nc.dram_tensor("w1t", [N_IN, N_HID], F32, kind="ExternalInput")
    wb_d = nc.dram_tensor("wb", [P, WB_COLS], F32, kind="ExternalInput")
    y_d = nc.dram_tensor("y", [n_rows_core, N_OUT], F32, kind="ExternalOutput")

    y_view = y_d[:].rearrange("(p i) o -> p (i o)", p=P)

    dve, gps = nc.vector, nc.gpsimd

    splits = []
    if gp_cols > 0:
        assert gp_cols % N_HID == 0 and 0 < gp_cols < free
        splits = [(dve, 0, free - gp_cols), (gps, free - gp_cols, gp_cols)]
    else:
        splits = [(dve, 0, free)]

    with tile.TileContext(nc) as tc:
        with tc.tile_pool(name="pool", bufs=1) as pool, \
             tc.tile_pool(name="ps", bufs=1, space="PSUM") as psp:
            xt = pool.tile([N_IN, n_rows_core], F32, tag="xt")
            nc.sync.dma_start(xt[:], xt_d[:])
            w1t = pool.tile([N_IN, N_HID], F32, tag="w1t")
            nc.sync.dma_start(w1t[:], w1t_d[:])
            wt = pool.tile([P, WB_COLS], F32, tag="wt")
            nc.sync.dma_start(wt[:], wb_d[:])

            cn = pool.tile([P, free], F32, tag="cn")   # -cur1
            nt = pool.tile([P, free], F32, tag="nt")   # scan state n = -mem
            at = pool.tile([P, free], F32, tag="at")   # scratch A
            ot = pool.tile([P, nch * N_OUT], F32, tag="ot")
            ht = pool.tile([P, max(gp_cols, N_HID)], F32, tag="ht")  # gpsimd scratch

            b1t = wt[:, B1_OFF : B1_OFF + 32]

            # --- cur1 via PE (bit-exact vs reference), negated+biased into cn ---
            # all chunk matmuls write one big PSUM tile, then a single STT
            # computes cn = (mm * -1) - b1 = -(mm + b1) = -cur1
            ps = psp.tile([P, free], F32, tag="psA")
            for ch in range(nch):
                nc.tensor.matmul(
                    ps[:, ch * N_HID : (ch + 1) * N_HID],
                    xt[:, ch * P : (ch + 1) * P], w1t[:],
                    start=True, stop=True,
                )
            b1b = b1t.unsqueeze(1).broadcast_to([P, nch, N_HID])
            dve.scalar_tensor_tensor(
                cn[:].rearrange("p (i h) -> p i h", h=N_HID),
                ps[:].rearrange("p (i h) -> p i h", h=N_HID),
                -1.0, b1b, ALU.mult, ALU.subtract,
            )
            # n = cn (membrane after step 1, negated)
            nc.scalar.copy(nt[:], cn[:])

            # --- scan steps 2..num_steps ---
            # DVE lane: 2 fused scalar_tensor_tensor ops per step.
            # Pool lane: STT is not in the Pool ISA; same values via
            # 4 plain ops (2 one-input tensor_scalar + 2 tensor_tensor).
            for _ in range(num_steps - 1):
                for eng, c0, cs in splits:
                    n_ap = nt[:, c0 : c0 + cs]
                    a_ap = at[:, c0 : c0 + cs]
                    c_ap = cn[:, c0 : c0 + cs]
                    if eng is dve:
                        eng.scalar_tensor_tensor(
                            a_ap, n_ap, -BETA, c_ap, ALU.mult, ALU.subtract
                        )
                        eng.scalar_tensor_tensor(
                            n_ap, n_ap, -THR, a_ap, ALU.is_lt, ALU.subtract
                        )
                    else:
                        h_ap = ht[:, 0:cs]
                        eng.tensor_scalar(a_ap, n_ap, -BETA, None, ALU.mult)
                        eng.tensor_tensor(a_ap, a_ap, c_ap, ALU.subtract)
                        eng.tensor_scalar(h_ap, n_ap, -THR, None, ALU.is_lt)
                        eng.tensor_tensor(n_ap, h_ap, a_ap, ALU.subtract)

            # --- spike + fc2 ---
            ov = ot[:].rearrange("p (i o) -> p o i", o=N_OUT)
            for eng, c0, cs in splits:
                eng.tensor_scalar(
                    at[:, c0 : c0 + cs], nt[:, c0 : c0 + cs], -THR, None, ALU.is_lt
                )
            for o in range(N_OUT):
                for eng, c0, cs in splits:
                    ib = cs // N_HID
                    i0 = c0 // N_HID
                    sv = at[:, c0 : c0 + cs].rearrange("p (i h) -> p i h", h=N_HID)
                    tv = cn[:, c0 : c0 + cs].rearrange("p (i h) -> p i h", h=N_HID)
                    w2o = (
                        wt[:, W2_OFF + 32 * o : W2_OFF + 32 * (o + 1)]
                        .unsqueeze(1)
                        .broadcast_to([P, ib, N_HID])
                    )
                    eng.tensor_tensor(tv, sv, w2o, ALU.mult)
                    dve.tensor_reduce(
                        ov[:, o : o + 1, i0 : i0 + ib], tv,
                        mybir.AxisListType.X, ALU.add,
                    )
                dve.tensor_scalar(
                    ov[:, o : o + 1, :], ov[:, o : o + 1, :],
                    wt[:, B2_OFF + o : B2_OFF + o + 1], None, ALU.add,
                )

            nc.sync.dma_start(y_view, ot[:])
    return nc


_CACHE = {}


def _get_program():
    if "nc" not in _CACHE:
        nc = bacc.Bacc("TRN2", target_bir_lowering=False, debug=False,
                       num_devices=N_CORES)
        build(nc)
        nc.compile()
        _CACHE["nc"] = nc
    return _CACHE["nc"]


def make_wb(b1, W2, b2):
    wb = np.zeros((P, WB_COLS), dtype=np.float32)
    wb[:, B1_OFF : B1_OFF + 32] = b1
    wb[:, W2_OFF : W2_OFF + 96] = np.ascontiguousarray(W2).reshape(-1)
    wb[:, B2_OFF : B2_OFF + 3] = b2
    return wb


def kernel(x, W1, b1, W2, b2):
    x = np.asarray(x, dtype=np.float32)
    W1, b1, W2, b2 = (np.asarray(a, dtype=np.float32) for a in (W1, b1, W2, b2))
    wb = make_wb(b1, W2, b2)
    w1t = np.ascontiguousarray(W1.T)
    nc = _get_program()
    in_maps = [
        {
            "xT": np.ascontiguousarray(x[i * BC : (i + 1) * BC].T),
            "w1t": w1t,
            "wb": wb,
        }
        for i in range(N_CORES)
    ]
    kwargs = dict(_CACHE.get("run_kwargs") or {})
    res = run_bass_kernel_spmd(nc, in_maps, core_ids=list(range(N_CORES)), **kwargs)
    _CACHE["last_results"] = res
    # y rows are stored permuted: dram row p*NCH + ch  <->  logical row ch*P + p
    out = np.empty((B, N_OUT), dtype=np.float32)
    for i in range(N_CORES):
        yc = res.results[i]["y"].reshape(P, NCH, N_OUT)
        out[i * BC : (i + 1) * BC] = yc.transpose(1, 0, 2).reshape(BC, N_OUT)
    return out


# revision 20
# speedup vs baseline: 2.7193x; 2.7193x over previous
"""Trainium2 Bass kernel for the SNN Leaky-Integrate-Fire problem.

Pipeline (per core, pure data-parallel over batch):
  cur1 = x @ W1.T + b1                        [B,32]  (PE fp32 matmul, bit-exact
                                                       vs the XLA-neuron reference)
  100x: mem = beta*mem + cur1 - H(mem-1)      (elementwise scan)
  spk  = H(mem - 1)
  out  = spk @ W2.T + b2                      [B,3]   (segmented reduce over h)

Numerics: the scan tracks n = -mem so each step is two fused
scalar_tensor_tensor ops (DVE lane) whose per-stage fp32 roundings match
the reference's  fl(fl(fl(beta*m)+cur1)-h)  sequence bit-for-bit:
  A  = (n * -beta) - cn         # cn = -cur1; A = fl(fl(beta*m)+cur1)
  n' = (n is_lt -1) - A         # n' = fl(h - A) = -m'
A second, independent column range runs on GPSIMD with the same values
via 4 plain tensor_tensor ops (STT is not in the Pool ISA and Pool
TENSOR_SCALAR is pathologically slow; TT with broadcast-constant views
is fast). Lanes use disjoint tiles so the engines never synchronize.

cur1 is computed on the PE with the exact operand layout the XLA-neuron
compiler uses for this matmul (stationary = x-chunk.T [3,128], moving =
W1.T [3,32], fp32 LOW/HIGH mode) -> bit-identical spikes.

Layout per core: 8192 rows; logical row r = chunk*128 + p lives at
partition p, free block chunk. Host feeds x_shard.T [3, 8192] and
inverse-permutes the output rows.
"""
import os
import sys

sys.path.insert(0, "/opt/trn_rl_repo")

import numpy as np

import concourse.bacc as bacc
import concourse.tile as tile
from concourse import mybir
from concourse.bass_utils import run_bass_kernel_spmd

F32 = mybir.dt.float32
ALU = mybir.AluOpType
AF = mybir.ActivationFunctionType

# problem constants (hardcoded per contract)
B, N_IN, N_HID, N_OUT = 65536, 3, 32, 3
NUM_STEPS, BETA, THR = 100, 0.9, 1.0
N_CORES = 8
BC = B // N_CORES          # rows per core = 8192
P = 128                    # partitions
NCH = BC // P              # 128-row chunks per core = 64
FREE = NCH * N_HID         # scan free size = 2048

# const block layout (replicated across partitions):
# [b1(32) w2(3*32) b2(3) pad(29) negbeta(32) negone(32)]
B1_OFF, W2_OFF, B2_OFF, NB_OFF, NO_OFF = 0, 32, 128, 160, 192
WB_COLS = 224

# scan columns handled by GPSIMD (0 = DVE only); must be a multiple of N_HID.
GP_COLS = int(os.environ.get("KERNEL_GP_COLS", "416"))
# 1 = ACT also does the beta-scale mult for the GPSIMD lane (pool: 2 TT/step)
ACT_MULT = int(os.environ.get("KERNEL_ACT_MULT", "0"))


def build(nc, n_rows_core=BC, num_steps=NUM_STEPS, gp_cols=GP_COLS):
    nch = n_rows_core // P
    free = nch * N_HID
    assert gp_cols % N_HID == 0 and 0 <= gp_cols < free
    dv_cols = free - gp_cols

    xt_d = nc.dram_tensor("xT", [N_IN, n_rows_core], F32, kind="ExternalInput")
    w1t_d = nc.dram_tensor("w1t", [N_IN, N_HID], F32, kind="ExternalInput")
    wb_d = nc.dram_tensor("wb", [P, WB_COLS], F32, kind="ExternalInput")
    y_d = nc.dram_tensor("y", [n_rows_core, N_OUT], F32, kind="ExternalOutput")

    y_view = y_d[:].rearrange("(p i) o -> p (i o)", p=P)

    dve, gps = nc.vector, nc.gpsimd

    with tile.TileContext(nc) as tc:
        with tc.tile_pool(name="pool", bufs=1) as pool, \
             tc.tile_pool(name="ps", bufs=1, space="PSUM") as psp:
            xt = pool.tile([N_IN, n_rows_core], F32, tag="xt")
            nc.sync.dma_start(xt[:], xt_d[:])
            w1t = pool.tile([N_IN, N_HID], F32, tag="w1t")
            nc.sync.dma_start(w1t[:], w1t_d[:])
            wt = pool.tile([P, WB_COLS], F32, tag="wt")
            nc.sync.dma_start(wt[:], wb_d[:])

            # per-lane state tiles: (cn, n, A[, h]) per engine lane
            lanes = []  # (eng, col0, ncols, cn, nt, at, ht)
            cn_d = pool.tile([P, dv_cols], F32, tag="cn_d")
            nt_d = pool.tile([P, dv_cols], F32, tag="nt_d")
            at_d = pool.tile([P, dv_cols], F32, tag="at_d")
            lanes.append((dve, 0, dv_cols, cn_d, nt_d, at_d, None))
            if gp_cols:
                cn_g = pool.tile([P, gp_cols], F32, tag="cn_g")
                nt_g = pool.tile([P, gp_cols], F32, tag="nt_g")
                at_g = pool.tile([P, gp_cols], F32, tag="at_g")
                ht_g = pool.tile([P, gp_cols], F32, tag="ht_g")
                lanes.append((gps, dv_cols, gp_cols, cn_g, nt_g, at_g, ht_g))

            ot = pool.tile([P, nch * N_OUT], F32, tag="ot")

            b1t = wt[:, B1_OFF : B1_OFF + 32]
            negone = wt[:, NO_OFF : NO_OFF + 1]

            def cbc(off, blocks):
                # [P, 32] const slice -> [P, blocks, 32] broadcast view
                return (
                    wt[:, off : off + 32].unsqueeze(1).broadcast_to([P, blocks, N_HID])
                )

            def h3(ap, cs):
                return ap.rearrange("p (i h) -> p i h", h=N_HID)

            # --- cur1 via PE (bit-exact vs reference), negated+biased into cn ---
            ps = psp.tile([P, free], F32, tag="psA")
            for ch in range(nch):
                nc.tensor.matmul(
                    ps[:, ch * N_HID : (ch + 1) * N_HID],
                    xt[:, ch * P : (ch + 1) * P], w1t[:],
                    start=True, stop=True,
                )
            # cn = (mm * -1) - b1 = -(mm + b1) = -cur1   (DVE reads PSUM)
            for eng, c0, cs, cn, nt, at, ht in lanes:
                ib = cs // N_HID
                dve.scalar_tensor_tensor(
                    h3(cn[:], cs), h3(ps[:, c0 : c0 + cs], cs), -1.0,
                    cbc(B1_OFF, ib), ALU.mult, ALU.subtract,
                )
                # n = cn (membrane after step 1, negated)
                nc.scalar.copy(nt[:], cn[:])

            # --- scan steps 2..num_steps ---
            for _ in range(num_steps - 1):
                for eng, c0, cs, cn, nt, at, ht in lanes:
                    ib = cs // N_HID
                    if eng is dve:
                        eng.scalar_tensor_tensor(
                            at[:], nt[:], -BETA, cn[:], ALU.mult, ALU.subtract
                        )
                        eng.scalar_tensor_tensor(
                            nt[:], nt[:], -THR, at[:], ALU.is_lt, ALU.subtract
                        )
                    else:
                        # h = [n < -1] = Relu(Sign(-n - 1)) on the (idle) ACT
                        # engine: Sign/Relu are exact (Sign(0)=0 verified on
                        # HW), so h is bit-exact. Pool does the arithmetic.
                        nc.scalar.activation(
                            ht[:], nt[:], AF.Sign, bias=negone, scale=-1.0
                        )
                        nc.scalar.activation(ht[:], ht[:], AF.Relu)
                        if ACT_MULT:
                            nc.scalar.mul(at[:], nt[:], -BETA)
                        else:
                            eng.tensor_tensor(
                                h3(at[:], cs), h3(nt[:], cs), cbc(NB_OFF, ib),
                                ALU.mult,
                            )
                        eng.tensor_tensor(at[:], at[:], cn[:], ALU.subtract)
                        eng.tensor_tensor(nt[:], ht[:], at[:], ALU.subtract)

            # --- spike + fc2 ---
            ov = ot[:].rearrange("p (i o) -> p o i", o=N_OUT)
            for eng, c0, cs, cn, nt, at, ht in lanes:
                ib = cs // N_HID
                if eng is dve:
                    eng.tensor_scalar(at[:], nt[:], -THR, None, ALU.is_lt)
                else:
                    nc.scalar.activation(
                        at[:], nt[:], AF.Sign, bias=negone, scale=-1.0
                    )
                    nc.scalar.activation(at[:], at[:], AF.Relu)
            for o in range(N_OUT):
                for eng, c0, cs, cn, nt, at, ht in lanes:
                    ib = cs // N_HID
                    i0 = c0 // N_HID
                    eng.tensor_tensor(
                        h3(cn[:], cs), h3(at[:], cs), cbc(W2_OFF + 32 * o, ib),
                        ALU.mult,
                    )
                    dve.tensor_reduce(
                        ov[:, o : o + 1, i0 : i0 + ib], h3(cn[:], cs),
                        mybir.AxisListType.X, ALU.add,
                    )
                dve.tensor_scalar(
                    ov[:, o : o + 1, :], ov[:, o : o + 1, :],
                    wt[:, B2_OFF + o : B2_OFF + o + 1], None, ALU.add,
                )

            nc.sync.dma_start(y_view, ot[:])
    return nc


_CACHE = {}


def _get_program():
    if "nc" not in _CACHE:
        nc = bacc.Bacc("TRN2", target_bir_lowering=False, debug=False,
                       num_devices=N_CORES)
        build(nc)
        nc.compile()
        _CACHE["nc"] = nc
    return _CACHE["nc"]


def make_wb(b1, W2, b2):
    wb = np.zeros((P, WB_COLS), dtype=np.float32)
    wb[:, B1_OFF : B1_OFF + 32] = b1
    wb[:, W2_OFF : W2_OFF + 96] = np.ascontiguousarray(W2).reshape(-1)
    wb[:, B2_OFF : B2_OFF + 3] = b2
    wb[:, NB_OFF : NB_OFF + 32] = np.float32(-BETA)
    wb[:, NO_OFF : NO_OFF + 32] = np.float32(-THR)
    return wb


def kernel(x, W1, b1, W2, b2):
    x = np.asarray(x, dtype=np.float32)
    W1, b1, W2, b2 = (np.asarray(a, dtype=np.float32) for a in (W1, b1, W2, b2))
    wb = make_wb(b1, W2, b2)
    w1t = np.ascontiguousarray(W1.T)
    nc = _get_program()
    in_maps = [
        {
            "xT": np.ascontiguousarray(x[i * BC : (i + 1) * BC].T),
            "w1t": w1t,
            "wb": wb,
        }
        for i in range(N_CORES)
    ]
    kwargs = dict(_CACHE.get("run_kwargs") or {})
    res = run_bass_kernel_spmd(nc, in_maps, core_ids=list(range(N_CORES)), **kwargs)
    _CACHE["last_results"] = res
    # y rows are stored permuted: dram row p*NCH + ch  <->  logical row ch*P + p
    out = np.empty((B, N_OUT), dtype=np.float32)
    for i in range(N_CORES):
        yc = res.results[i]["y"].reshape(P, NCH, N_OUT)
        out[i * BC : (i + 1) * BC] = yc.transpose(1, 0, 2).reshape(BC, N_OUT)
    return out
